# revision 5
# baseline (speedup 1.0000x reference)
"""Trainium2 Bass kernel: row-wise cosine similarity discriminator.

Computes, for full inputs s, h_rl, h_fk of shape [B=8, N=8192, D=512] f32:
    out = concat(rowdot(l2n(s), l2n(h_rl)), rowdot(l2n(s), l2n(h_fk)), axis=1)
with l2n(x) = x / max(||x||_2, 1e-12), giving out shape [8, 16384] f32.

Sharding: pure data parallel over batch B — core b processes batch b.

Per-core kernel strategy (memory-bound: 48 MiB input / core, HBM limit
~358 GB/s per core -> ~141 us DMA floor):
  - row mapping row = p*NT + t: partition p holds NT=64 consecutive DRAM
    rows, so chunked loads [128, J, 512] are J*2 KiB CONTIGUOUS per
    partition (large DMA descriptors), and the final stats tile [P, NT]
    stores to DRAM directly with no transpose (out[k] viewed as [P, NT]).
  - engine split (HW-measured: DVE tensor_reduce only has a 1x uop ->
    4332ns per [P,4096]; ACT Square+accum_out on [P,512] is 720+186ns,
    a fused square+reduce with no DVE work), everything under the floor:
      ACT    s^2 and h_rl^2 as per-row-tile Square+accum_out (fused)
      GpSimd p_rl, p_fk products and h_fk^2 (f32 in -> bf16 scratch)
      DVE    3 batched reductions of the gpsimd outputs (1x mode)
  - variable chunk sizes (small first chunks for fast pipeline ramp,
    small last chunks for a short tail after the final DMA)
  - finals (sqrt on ACT; clamp/reciprocal/scale on DVE — gpsimd costs
    ~1us per tiny op) on [128, 64] stats tiles
  - this walrus build cannot encode multi-wait Drain/STT instructions:
    _fix_tail_drain_waits() rewrites multi-wait instructions into
    single-wait EventSemaphores
"""

import numpy as np

import concourse.bass as bass
import concourse.mybir as mybir
import concourse.tile as tile
from concourse.bass_utils import run_bass_kernel_spmd

B, N, D = 8, 8192, 512
P = 128                    # SBUF partitions
NT = N // P                # 64 rows per partition (row = p*NT + t)
JMAX = 8                   # max row-tiles per chunk ([P, J, D] per DMA)
# chunk sizes: fast ramp, big middle, short tail; sums to NT
CHUNKS = [1, 1, 2, 4] + [8] * 6 + [4, 2, 1, 1]
assert sum(CHUNKS) == NT
EPS = 1e-12
F32 = mybir.dt.float32
BF16 = mybir.dt.bfloat16


def _fix_tail_drain_waits(nc):
    """This image's walrus cannot encode more than one sem wait on several
    instruction kinds (Tile's end-of-kernel Drain, STT, ...). Move each
    wait of any multi-wait instruction onto its own EventSemaphore
    inserted right before it on the same engine — identical semantics
    (engine program order), always encodable."""
    for fn in nc.m.functions:
        for bb in fn.blocks:
            new = []
            for inst in bb.instructions:
                si = inst.sync_info
                if (
                    not isinstance(inst, mybir.InstEventSemaphore)
                    and si is not None
                    and si.on_wait
                    and len(si.on_wait) > 1
                ):
                    for k, w in enumerate(list(si.on_wait)):
                        ev = mybir.InstEventSemaphore(
                            name=f"{inst.name}-prewait{k}", ins=[], outs=[]
                        )
                        ev.engine = inst.engine
                        ev.sync_info = mybir.SyncInfo(on_wait=[w], on_update=[])
                        new.append(ev)
                    inst.sync_info = mybir.SyncInfo(
                        on_wait=[], on_update=list(si.on_update)
                    )
                new.append(inst)
            bb.instructions[:] = new


def build_nc():
    nc = bass.Bass(trn_type="TRN2")
    s_h = nc.declare_dram_parameter("s", [N, D], F32, isOutput=False)
    hrl_h = nc.declare_dram_parameter("h_rl", [N, D], F32, isOutput=False)
    hfk_h = nc.declare_dram_parameter("h_fk", [N, D], F32, isOutput=False)
    # out[k][p, t] = score of row p*NT + t  ->  flat [2, N] row-major
    out_h = nc.declare_dram_parameter("out", [2, P, NT], F32, isOutput=True)

    # DRAM view: row p*NT + t  ->  [p, t, d]; per-partition rows contiguous
    def rows(h):
        return h[:, :].rearrange("(p t) d -> p t d", p=P, t=NT)

    s_g, h1_g, h2_g = rows(s_h), rows(hrl_h), rows(hfk_h)

    Sq = mybir.ActivationFunctionType.Square
    Red = dict(axis=mybir.AxisListType.X, op=mybir.AluOpType.add)
    Mult = mybir.AluOpType.mult

    with tile.TileContext(nc) as tc:
        with (
            tc.tile_pool(name="ins", bufs=2) as ins,
            tc.tile_pool(name="scr", bufs=2) as scr,
            tc.tile_pool(name="stats", bufs=1) as stats,
            tc.tile_pool(name="fin", bufs=1) as fin,
        ):
            # per-row accumulators, column t = row's slot in its partition
            # stats_q: [ss, hh_rl, hh_fk]; stats_p: [sp_rl, sp_fk]
            stats_q = stats.tile([P, 3, NT], F32, tag="stats_q")
            stats_p = stats.tile([P, 2, NT], F32, tag="stats_p")
            ss, hh_rl, hh_fk = (stats_q[:, k, :] for k in range(3))
            sp_rl, sp_fk = (stats_p[:, k, :] for k in range(2))

            # junk sink for the fused squares' elementwise outputs (the
            # accum_out is what we keep); WAW on it is same-engine serial
            junk = fin.tile([P, D], BF16, tag="junk")

            t0 = 0
            for J in CHUNKS:
                cols = slice(t0, t0 + J)
                s_f = ins.tile([P, JMAX, D], F32, tag="s")
                h1_f = ins.tile([P, JMAX, D], F32, tag="h1")
                h2_f = ins.tile([P, JMAX, D], F32, tag="h2")
                q2_f = scr.tile([P, JMAX, D], BF16, tag="q2")
                p1_f = scr.tile([P, JMAX, D], BF16, tag="p1")
                p2_f = scr.tile([P, JMAX, D], BF16, tag="p2")
                s_t, h1_t, h2_t = s_f[:, :J, :], h1_f[:, :J, :], h2_f[:, :J, :]
                q2, p1, p2 = q2_f[:, :J, :], p1_f[:, :J, :], p2_f[:, :J, :]

                nc.sync.dma_start(out=s_t, in_=s_g[:, cols, :])
                nc.sync.dma_start(out=h1_t, in_=h1_g[:, cols, :])
                nc.sync.dma_start(out=h2_t, in_=h2_g[:, cols, :])

                # ACT: fused square+reduce per row-tile (s^2 then h_rl^2,
                # in DMA arrival order)
                for j in range(J):
                    nc.scalar.activation(
                        out=junk, in_=s_t[:, j, :], func=Sq,
                        accum_out=stats_q[:, 0, t0 + j:t0 + j + 1])
                for j in range(J):
                    nc.scalar.activation(
                        out=junk, in_=h1_t[:, j, :], func=Sq,
                        accum_out=stats_q[:, 1, t0 + j:t0 + j + 1])

                # GpSimd: both products and the h_fk square -> bf16
                nc.gpsimd.tensor_tensor(out=p1, in0=s_t, in1=h1_t, op=Mult)
                nc.gpsimd.tensor_tensor(out=q2, in0=h2_t, in1=h2_t, op=Mult)
                nc.gpsimd.tensor_tensor(out=p2, in0=s_t, in1=h2_t, op=Mult)

                # DVE: batched reductions of the gpsimd outputs
                nc.vector.tensor_reduce(out=stats_p[:, 0, cols], in_=p1, **Red)
                nc.vector.tensor_reduce(out=stats_q[:, 2, cols], in_=q2, **Red)
                nc.vector.tensor_reduce(out=stats_p[:, 1, cols], in_=p2, **Red)
                t0 += J

            # ---- finals on [P, NT] stats tiles (sqrt on ACT; everything
            # else on DVE — reciprocal must be DVE anyway, and gpsimd
            # costs ~1us per tiny op) ----
            Sqrt = mybir.ActivationFunctionType.Sqrt
            ns = fin.tile([P, NT], F32, tag="ns")
            n1 = fin.tile([P, NT], F32, tag="n1")
            n2 = fin.tile([P, NT], F32, tag="n2")
            nc.scalar.activation(out=ns, in_=ss, func=Sqrt)
            nc.scalar.activation(out=n1, in_=hh_rl, func=Sqrt)
            nc.scalar.activation(out=n2, in_=hh_fk, func=Sqrt)
            nc.vector.tensor_scalar_max(ns, ns, EPS)
            nc.vector.tensor_scalar_max(n1, n1, EPS)
            nc.vector.tensor_scalar_max(n2, n2, EPS)
            den1 = fin.tile([P, NT], F32, tag="den1")
            den2 = fin.tile([P, NT], F32, tag="den2")
            nc.vector.tensor_tensor(den1, ns, n1, op=Mult)
            nc.vector.tensor_tensor(den2, ns, n2, op=Mult)
            nc.vector.reciprocal(den1, den1)
            nc.vector.reciprocal(den2, den2)
            o1 = fin.tile([P, NT], F32, tag="o1")
            o2 = fin.tile([P, NT], F32, tag="o2")
            nc.vector.tensor_tensor(o1, sp_rl, den1, op=Mult)
            nc.vector.tensor_tensor(o2, sp_fk, den2, op=Mult)
            nc.sync.dma_start(out=out_h[0], in_=o1)
            nc.sync.dma_start(out=out_h[1], in_=o2)

    _fix_tail_drain_waits(nc)
    return nc


_NC_CACHE = None


def kernel(s, h_rl, h_fk, trace=False):
    global _NC_CACHE
    s = np.ascontiguousarray(np.asarray(s, dtype=np.float32))
    h_rl = np.ascontiguousarray(np.asarray(h_rl, dtype=np.float32))
    h_fk = np.ascontiguousarray(np.asarray(h_fk, dtype=np.float32))
    assert s.shape == (B, N, D), s.shape

    if _NC_CACHE is None:
        _NC_CACHE = build_nc()
    nc = _NC_CACHE

    in_maps = [
        {"s": s[b], "h_rl": h_rl[b], "h_fk": h_fk[b]} for b in range(B)
    ]
    res = run_bass_kernel_spmd(nc, in_maps, core_ids=list(range(B)), trace=trace)
    out = np.empty((B, 2 * N), dtype=np.float32)
    for b in range(B):
        o = res.results[b]["out"]  # [2, P, NT]; row p*NT+t -> o[k].ravel()
        out[b, :N] = o[0].reshape(N)
        out[b, N:] = o[1].reshape(N)
    if trace:
        return out, res
    return out


# revision 8
# speedup vs baseline: 1.1852x; 1.1852x over previous
"""Trainium2 Bass kernel: row-wise cosine similarity discriminator.

Computes, for full inputs s, h_rl, h_fk of shape [B=8, N=8192, D=512] f32:
    out = concat(rowdot(l2n(s), l2n(h_rl)), rowdot(l2n(s), l2n(h_fk)), axis=1)
with l2n(x) = x / max(||x||_2, 1e-12), giving out shape [8, 16384] f32.

Sharding: pure data parallel over batch B — core b processes batch b.

Per-core kernel strategy (memory-bound: 48 MiB input / core, HBM limit
~358 GB/s per core -> ~141 us DMA floor):
  - row mapping row = p*NT + t: partition p holds NT=64 consecutive DRAM
    rows, so chunked loads [128, J, 512] are J*2 KiB CONTIGUOUS per
    partition (large DMA descriptors), and the final stats tile [P, NT]
    stores to DRAM directly with no transpose (out[k] viewed as [P, NT]).
  - engine split (HW-measured per-row-tile costs: ACT fused Square+
    accum_out 906ns, ACT batched square 463ns, GpSimd tensor_tensor
    1100ns, DVE tensor_tensor 553ns, DVE batched reduce 541ns; DVE
    tensor_reduce only has a 1x uop; gpsimd is ~2.15 ns/elem, its
    architectural floor), min-max balanced at ~2.1us/tile/engine:
      ACT    s^2 and h_rl^2 fused Square+accum_out; last 1/4 of the
             h_fk^2 tiles as batched squares
      GpSimd p_rl = s*h_rl; first 3/4 of the h_fk^2 tiles
      DVE    p_fk = s*h_fk; all 3 batched reductions (p_rl, hh_fk, p_fk)
  - variable chunk sizes (small first chunks for fast pipeline ramp,
    small last chunks for a short tail after the final DMA)
  - finals (sqrt on ACT; clamp/reciprocal/scale on DVE — gpsimd costs
    ~1us per tiny op) on [128, 64] stats tiles
  - this walrus build cannot encode multi-wait Drain/STT instructions:
    _fix_tail_drain_waits() rewrites multi-wait instructions into
    single-wait EventSemaphores
"""

import numpy as np

import concourse.bass as bass
import concourse.mybir as mybir
import concourse.tile as tile
from concourse.bass_utils import run_bass_kernel_spmd

B, N, D = 8, 8192, 512
P = 128                    # SBUF partitions
NT = N // P                # 64 rows per partition (row = p*NT + t)
JMAX = 8                   # max row-tiles per chunk ([P, J, D] per DMA)
# chunk sizes: fast ramp, big middle, short tail; sums to NT
CHUNKS = [2, 2, 4] + [8] * 6 + [4, 2, 1, 1]
assert sum(CHUNKS) == NT
EPS = 1e-12
F32 = mybir.dt.float32
FP16 = mybir.dt.float16


def _fix_tail_drain_waits(nc):
    """This image's walrus cannot encode more than one sem wait on several
    instruction kinds (Tile's end-of-kernel Drain, STT, ...). Move each
    wait of any multi-wait instruction onto its own EventSemaphore
    inserted right before it on the same engine — identical semantics
    (engine program order), always encodable."""
    for fn in nc.m.functions:
        for bb in fn.blocks:
            new = []
            for inst in bb.instructions:
                si = inst.sync_info
                if (
                    not isinstance(inst, mybir.InstEventSemaphore)
                    and si is not None
                    and si.on_wait
                    and len(si.on_wait) > 1
                ):
                    for k, w in enumerate(list(si.on_wait)):
                        ev = mybir.InstEventSemaphore(
                            name=f"{inst.name}-prewait{k}", ins=[], outs=[]
                        )
                        ev.engine = inst.engine
                        ev.sync_info = mybir.SyncInfo(on_wait=[w], on_update=[])
                        new.append(ev)
                    inst.sync_info = mybir.SyncInfo(
                        on_wait=[], on_update=list(si.on_update)
                    )
                new.append(inst)
            bb.instructions[:] = new


def build_nc():
    nc = bass.Bass(trn_type="TRN2")
    s_h = nc.declare_dram_parameter("s", [N, D], F32, isOutput=False)
    hrl_h = nc.declare_dram_parameter("h_rl", [N, D], F32, isOutput=False)
    hfk_h = nc.declare_dram_parameter("h_fk", [N, D], F32, isOutput=False)
    # out[k][p, t] = score of row p*NT + t  ->  flat [2, N] row-major
    out_h = nc.declare_dram_parameter("out", [2, P, NT], F32, isOutput=True)

    # DRAM view: row p*NT + t  ->  [p, t, d]; per-partition rows contiguous
    def rows(h):
        return h[:, :].rearrange("(p t) d -> p t d", p=P, t=NT)

    s_g, h1_g, h2_g = rows(s_h), rows(hrl_h), rows(hfk_h)

    Sq = mybir.ActivationFunctionType.Square
    Red = dict(axis=mybir.AxisListType.X, op=mybir.AluOpType.add)
    Mult = mybir.AluOpType.mult

    with tile.TileContext(nc) as tc:
        with (
            tc.tile_pool(name="ins", bufs=2) as ins,
            tc.tile_pool(name="scr", bufs=2) as scr,
            tc.tile_pool(name="stats", bufs=1) as stats,
            tc.tile_pool(name="fin", bufs=1) as fin,
        ):
            # per-row accumulators, column t = row's slot in its partition
            # stats_q: [ss, hh_rl, hh_fk]; stats_p: [sp_rl, sp_fk]
            stats_q = stats.tile([P, 3, NT], F32, tag="stats_q")
            stats_p = stats.tile([P, 2, NT], F32, tag="stats_p")
            ss, hh_rl, hh_fk = (stats_q[:, k, :] for k in range(3))
            sp_rl, sp_fk = (stats_p[:, k, :] for k in range(2))

            # junk sink for the fused squares' elementwise outputs (the
            # accum_out is what we keep); WAW on it is same-engine serial
            junk = fin.tile([P, D], FP16, tag="junk")

            t0 = 0
            for J in CHUNKS:
                cols = slice(t0, t0 + J)
                s_f = ins.tile([P, JMAX, D], F32, tag="s")
                h1_f = ins.tile([P, JMAX, D], F32, tag="h1")
                h2_f = ins.tile([P, JMAX, D], F32, tag="h2")
                q2_f = scr.tile([P, JMAX, D], FP16, tag="q2")
                p1_f = scr.tile([P, JMAX, D], FP16, tag="p1")
                p2_f = scr.tile([P, JMAX, D], FP16, tag="p2")
                s_t, h1_t, h2_t = s_f[:, :J, :], h1_f[:, :J, :], h2_f[:, :J, :]
                q2, p1, p2 = q2_f[:, :J, :], p1_f[:, :J, :], p2_f[:, :J, :]

                nc.sync.dma_start(out=s_t, in_=s_g[:, cols, :])
                nc.sync.dma_start(out=h1_t, in_=h1_g[:, cols, :])
                nc.sync.dma_start(out=h2_t, in_=h2_g[:, cols, :])

                # ACT: fused square+reduce per row-tile (s^2 then h_rl^2,
                # in DMA arrival order)
                for j in range(J):
                    nc.scalar.activation(
                        out=junk, in_=s_t[:, j, :], func=Sq,
                        accum_out=stats_q[:, 0, t0 + j:t0 + j + 1])
                for j in range(J):
                    nc.scalar.activation(
                        out=junk, in_=h1_t[:, j, :], func=Sq,
                        accum_out=stats_q[:, 1, t0 + j:t0 + j + 1])

                # h_fk^2 split: first js tiles on gpsimd, rest batched on ACT
                js = (3 * J) // 4
                # GpSimd: p_rl product, then its share of h_fk^2
                nc.gpsimd.tensor_tensor(out=p1, in0=s_t, in1=h1_t, op=Mult)
                if js > 0:
                    nc.gpsimd.tensor_tensor(
                        out=q2[:, :js, :], in0=h2_t[:, :js, :],
                        in1=h2_t[:, :js, :], op=Mult)
                if js < J:
                    nc.scalar.activation(
                        out=q2[:, js:, :], in_=h2_t[:, js:, :], func=Sq)

                # DVE: p_fk product + the 3 batched reductions
                nc.vector.tensor_tensor(out=p2, in0=s_t, in1=h2_t, op=Mult)
                nc.vector.tensor_reduce(out=stats_p[:, 0, cols], in_=p1, **Red)
                nc.vector.tensor_reduce(out=stats_q[:, 2, cols], in_=q2, **Red)
                nc.vector.tensor_reduce(out=stats_p[:, 1, cols], in_=p2, **Red)
                t0 += J

            # ---- finals on [P, NT] stats tiles (sqrt on ACT; everything
            # else on DVE — reciprocal must be DVE anyway, and gpsimd
            # costs ~1us per tiny op) ----
            Sqrt = mybir.ActivationFunctionType.Sqrt
            ns = fin.tile([P, NT], F32, tag="ns")
            n1 = fin.tile([P, NT], F32, tag="n1")
            n2 = fin.tile([P, NT], F32, tag="n2")
            nc.scalar.activation(out=ns, in_=ss, func=Sqrt)
            nc.scalar.activation(out=n1, in_=hh_rl, func=Sqrt)
            nc.scalar.activation(out=n2, in_=hh_fk, func=Sqrt)
            nc.vector.tensor_scalar_max(ns, ns, EPS)
            nc.vector.tensor_scalar_max(n1, n1, EPS)
            nc.vector.tensor_scalar_max(n2, n2, EPS)
            den1 = fin.tile([P, NT], F32, tag="den1")
            den2 = fin.tile([P, NT], F32, tag="den2")
            nc.vector.tensor_tensor(den1, ns, n1, op=Mult)
            nc.vector.tensor_tensor(den2, ns, n2, op=Mult)
            nc.vector.reciprocal(den1, den1)
            nc.vector.reciprocal(den2, den2)
            o1 = fin.tile([P, NT], F32, tag="o1")
            o2 = fin.tile([P, NT], F32, tag="o2")
            nc.vector.tensor_tensor(o1, sp_rl, den1, op=Mult)
            nc.vector.tensor_tensor(o2, sp_fk, den2, op=Mult)
            nc.sync.dma_start(out=out_h[0], in_=o1)
            nc.sync.dma_start(out=out_h[1], in_=o2)

    _fix_tail_drain_waits(nc)
    return nc


_NC_CACHE = None


def kernel(s, h_rl, h_fk, trace=False):
    global _NC_CACHE
    s = np.ascontiguousarray(np.asarray(s, dtype=np.float32))
    h_rl = np.ascontiguousarray(np.asarray(h_rl, dtype=np.float32))
    h_fk = np.ascontiguousarray(np.asarray(h_fk, dtype=np.float32))
    assert s.shape == (B, N, D), s.shape

    if _NC_CACHE is None:
        _NC_CACHE = build_nc()
    nc = _NC_CACHE

    in_maps = [
        {"s": s[b], "h_rl": h_rl[b], "h_fk": h_fk[b]} for b in range(B)
    ]
    res = run_bass_kernel_spmd(nc, in_maps, core_ids=list(range(B)), trace=trace)
    out = np.empty((B, 2 * N), dtype=np.float32)
    for b in range(B):
        o = res.results[b]["out"]  # [2, P, NT]; row p*NT+t -> o[k].ravel()
        out[b, :N] = o[0].reshape(N)
        out[b, N:] = o[1].reshape(N)
    if trace:
        return out, res
    return out
